# revision 34
# baseline (speedup 1.0000x reference)
"""Trainium2 Bass kernel for the Discriminator GNN message-passing problem.

Computation (per reference):
  pos_score  = rowsum((node_emb[pos_src]  @ rel[r]) * node_emb[pos_dst])   per r, concat
  neg_score1 = rowsum((node_emb[neg1_src] @ rel[r]) * node_emb[neg1_dst])  per r, concat
  neg_score2 = rowsum((node_emb[neg2_src] @ rel[r]) * fake_f[r])           per r, concat
  graph_embd = relu(mean(node_emb, 0) @ w1 + b1) @ w2 + b2

Sharding: edges (E axis) split across 8 cores; node_emb + rel replicated.
Per core, per (comp, r) group of ES=3750 edges, tiles of ETILE=125 edges:
  - indirect-DMA gather of src rows (and dst rows for pos/neg1), one 125-row
    gather per tile (HW indirect DMA consumes one index per partition)
  - DVE cast of the gathered rows to bf16
  - PE transpose (bf16) of [125, 572] tiles into [572, 125] chunks in PSUM
  - PSUM->SBUF copies (DVE for the 4 full chunks, ACT for the 60-row tail)
  - 2x5 chunked bf16 matmuls vs rel[r] -> s [125, 572] in PSUM (fp32)
  - fused dot via custom-DVE TENSOR_TENSOR_REDUCE: score = rowsum(s * other)
The graph-embedding mean runs as an accumulating fp32 ones-matmul; the host
reduces per-core partials and applies the 2-layer MLP (trivial FLOPs).
"""

import os

import numpy as np

# ---- problem constants (hardcoded per spec; kernel.py must be self-contained)
N_NODES = 50000
E = 30000
R = 6
D = 572
NCORES = 8
ES = E // NCORES  # 3750 edges per relation per core
ETILE = 125  # edges per tile (partition dim)
NTILES = ES // ETILE  # 30
GB = 3  # tiles per gather/processing batch
D0 = 512  # free-dim split of D for PSUM banks
D1 = D - D0  # 60
KCHUNKS = [(0, 128), (128, 128), (256, 128), (384, 128), (512, 60)]
MEAN_ROWS = N_NODES // NCORES  # 6250 rows summed per core

_BUILD_CACHE = {}


def build_kernel(
    n_nodes=N_NODES,
    es=ES,
    ntiles=NTILES,
    etile=ETILE,
    gb=GB,
    mean_rows=MEAN_ROWS,
    mode="full",  # "full" | "meanonly" | combos of "nogather","nomm","nottr"
    reps=1,  # repeat the whole computation (for slope-based benchmarking)
):
    """Build the Bass module. Returns nc."""
    import concourse.bass as bass
    import concourse.mybir as mybir
    from concourse import bacc
    from concourse.dve_ops import TENSOR_TENSOR_REDUCE as TTR_OP
    from concourse.masks import make_identity
    from concourse.tile import TileContext

    f32 = mybir.dt.float32
    bf16 = mybir.dt.bfloat16
    i32 = mybir.dt.int32

    assert ntiles % gb == 0

    nc = bacc.Bacc()

    node_emb = nc.declare_dram_parameter("node_emb", [n_nodes, D], f32, isOutput=False)
    relw = nc.declare_dram_parameter("relw", [R, D, D], bf16, isOutput=False)
    fake = nc.declare_dram_parameter("fake", [R, es, D], f32, isOutput=False)
    # 5 index planes: pos_src, pos_dst, neg1_src, neg1_dst, neg2_src.
    # Host pre-transposes each group's indices to [etile, ntiles].
    idx = nc.declare_dram_parameter("idx", [5, R, etile, ntiles], i32, isOutput=False)
    node_slice = nc.declare_dram_parameter(
        "node_slice", [mean_rows, D], f32, isOutput=False
    )
    scores = nc.declare_dram_parameter(
        "scores", [3, R, etile, ntiles], f32, isOutput=True
    )
    mean_partial = nc.declare_dram_parameter("mean_partial", [1, D], f32, isOutput=True)

    with TileContext(nc) as tc:
        with (
            tc.tile_pool(name="const", bufs=1) as constp,
            tc.tile_pool(name="gath", bufs=3) as gathp,
            tc.tile_pool(name="ht", bufs=3) as htp,
            tc.tile_pool(name="pst", bufs=2, space="PSUM") as pstp,
            tc.tile_pool(name="pss", bufs=2, space="PSUM") as pssp,
            tc.tile_pool(name="meanps", bufs=1, space="PSUM") as meanpsp,
            tc.tile_pool(name="outp", bufs=2) as outp,
            tc.tile_pool(name="misc", bufs=2) as miscp,
        ):
            identity = constp.tile([128, 128], bf16, tag="ident")
            make_identity(nc, identity[:])
            id_b = identity[:etile, :etile]

            # rel weights resident in SBUF, bf16, chunked along K:
            # m_sb[r][:kw, c*D:(c+1)*D] = rel[r][k0:k0+kw, :]
            m_sb = []
            for r in range(R):
                mt = constp.tile([128, len(KCHUNKS) * D], bf16, tag=f"m{r}")
                for c, (k0, kw) in enumerate(KCHUNKS):
                    nc.sync.dma_start(
                        out=mt[:kw, c * D : (c + 1) * D], in_=relw[r, k0 : k0 + kw, :]
                    )
                m_sb.append(mt)

            ones = constp.tile([128, 1], f32, tag="ones")
            nc.gpsimd.memset(ones[:, :], 1.0)
            msum = constp.tile([1, D], f32, tag="msum")

            for _rep in range(reps):
                # graph-embedding partial sum via accumulating ones-matmul:
                # mean_partial[0, :] = sum over node_slice rows (fp32 MACs)
                assert mean_rows >= 128
                ps_m1 = meanpsp.tile([1, D0], f32, tag="m1")
                ps_m2 = meanpsp.tile([1, D1], f32, tag="m2")
                MB = 8  # row-tiles per mean DMA
                row_tiles = []
                r0 = 0
                while r0 < mean_rows:
                    rw = min(128 * MB, mean_rows - r0)
                    row_tiles.append((r0, rw))
                    r0 += rw
                for bi, (r0, rw) in enumerate(row_tiles):
                    mtile = miscp.tile([128, MB * D], f32, tag="mean_in")
                    full = rw // 128  # full 128-row subtiles in this DMA
                    if full:
                        src = node_slice[r0 : r0 + full * 128, :].rearrange(
                            "(a p) d -> p a d", p=128
                        )
                        dst3 = mtile[:, : full * D].rearrange("p (a d) -> p a d", d=D)
                        nc.sync.dma_start(out=dst3, in_=src)
                    tail = rw - full * 128
                    if tail:
                        nc.sync.dma_start(
                            out=mtile[:tail, full * D : (full + 1) * D],
                            in_=node_slice[r0 + full * 128 : r0 + rw, :],
                        )
                    n_sub = full + (1 if tail else 0)
                    for a in range(n_sub):
                        kw = 128 if a < full else tail
                        first = bi == 0 and a == 0
                        last = bi == len(row_tiles) - 1 and a == n_sub - 1
                        nc.tensor.matmul(
                            ps_m1[:1, :],
                            ones[:kw, :],
                            mtile[:kw, a * D : a * D + D0],
                            start=first,
                            stop=last,
                        )
                        nc.tensor.matmul(
                            ps_m2[:1, :],
                            ones[:kw, :],
                            mtile[:kw, a * D + D0 : (a + 1) * D],
                            start=first,
                            stop=last,
                        )
                nc.vector.tensor_copy(out=msum[:1, :D0], in_=ps_m1[:1, :])
                nc.vector.tensor_copy(out=msum[:1, D0:], in_=ps_m2[:1, :])
                nc.sync.dma_start(out=mean_partial[:1, :], in_=msum[:1, :])

                # main edge loops
                for comp in range(3 if mode != "meanonly" else 0):
                    si = [0, 2, 4][comp]
                    di = [1, 3, None][comp]
                    for r in range(R):
                        idx_s = miscp.tile([128, ntiles], i32, tag="idxs")
                        nc.sync.dma_start(out=idx_s[:etile, :], in_=idx[si, r])
                        if di is not None:
                            idx_d = miscp.tile([128, ntiles], i32, tag="idxd")
                            nc.sync.dma_start(out=idx_d[:etile, :], in_=idx[di, r])
                        scores_sb = outp.tile([128, ntiles], f32, tag="scores")

                        for bt in range(ntiles // gb):
                            h_raw = gathp.tile([128, gb * D], f32, tag="h")
                            if "nogather" in mode:
                                e0 = bt * gb * etile
                                src = node_emb[e0 : e0 + gb * etile, :].rearrange(
                                    "(j p) d -> p j d", p=etile
                                )
                                nc.sync.dma_start(
                                    out=h_raw[:etile, :].rearrange(
                                        "p (j d) -> p j d", d=D
                                    ),
                                    in_=src,
                                )
                            else:
                                # HW indirect DMA consumes ONE index per
                                # partition: one 125-row gather per tile
                                for j in range(gb):
                                    t = bt * gb + j
                                    nc.gpsimd.indirect_dma_start(
                                        out=h_raw[:etile, j * D : (j + 1) * D],
                                        out_offset=None,
                                        in_=node_emb[:, :],
                                        in_offset=bass.IndirectOffsetOnAxis(
                                            ap=idx_s[:etile, t : t + 1], axis=0
                                        ),
                                    )
                            o_raw = gathp.tile([128, gb * D], f32, tag="o")
                            if di is not None and "nogather" in mode:
                                e0 = bt * gb * etile
                                src = node_emb[e0 : e0 + gb * etile, :].rearrange(
                                    "(j p) d -> p j d", p=etile
                                )
                                nc.sync.dma_start(
                                    out=o_raw[:etile, :].rearrange(
                                        "p (j d) -> p j d", d=D
                                    ),
                                    in_=src,
                                )
                            elif di is not None:
                                for j in range(gb):
                                    t = bt * gb + j
                                    nc.gpsimd.indirect_dma_start(
                                        out=o_raw[:etile, j * D : (j + 1) * D],
                                        out_offset=None,
                                        in_=node_emb[:, :],
                                        in_offset=bass.IndirectOffsetOnAxis(
                                            ap=idx_d[:etile, t : t + 1], axis=0
                                        ),
                                    )
                            else:
                                e0 = bt * gb * etile
                                src = fake[r, e0 : e0 + gb * etile, :].rearrange(
                                    "(j p) d -> p j d", p=etile
                                )
                                dst3 = o_raw[:etile, :].rearrange(
                                    "p (j d) -> p j d", d=D
                                )
                                nc.sync.dma_start(out=dst3, in_=src)

                            # cast the gathered batch to bf16 once (DVE)
                            h_bf = gathp.tile([128, gb * D], bf16, tag="hb")
                            nc.vector.tensor_copy(
                                out=h_bf[:etile, :], in_=h_raw[:etile, :]
                            )

                            for j in range(gb):
                                t = bt * gb + j
                                hsl = h_bf[:etile, j * D : (j + 1) * D]
                                osl = o_raw[:etile, j * D : (j + 1) * D]

                                # transpose h tile: 5 chunks -> one psum bank
                                # (bf16); chunks strided by 128 elems so each
                                # start is 4B-aligned
                                CS = 128
                                ps_t = pstp.tile([128, 5 * CS], bf16, tag="pst")
                                for c, (k0, kw) in enumerate(KCHUNKS):
                                    nc.tensor.transpose(
                                        out=ps_t[:kw, c * CS : c * CS + etile],
                                        in_=hsl[:, k0 : k0 + kw],
                                        identity=id_b,
                                    )

                                # psum -> sbuf copies: big one on DVE (strided
                                # 3D views skip the alignment gaps), small on
                                # ACT
                                ht = htp.tile(
                                    [128, len(KCHUNKS) * CS], bf16, tag="ht"
                                )
                                in3 = ps_t[:, : 4 * CS].rearrange(
                                    "p (c e) -> p c e", e=CS
                                )[:, :, :etile]
                                out3 = ht[:, : 4 * CS].rearrange(
                                    "p (c e) -> p c e", e=CS
                                )[:, :, :etile]
                                nc.vector.tensor_copy(out=out3, in_=in3)
                                nc.scalar.copy(
                                    out=ht[:60, 4 * CS : 4 * CS + etile],
                                    in_=ps_t[:60, 4 * CS : 4 * CS + etile],
                                )

                                if "nomm" in mode:
                                    nc.vector.tensor_copy(
                                        out=scores_sb[:etile, t : t + 1],
                                        in_=ht[:etile, :1],
                                    )
                                    continue

                                # matmuls: s[e, f] accumulated over 5 K-chunks
                                ps_s1 = pssp.tile([128, D0], f32, tag="s1")
                                ps_s2 = pssp.tile([128, D1], f32, tag="s2")
                                for c, (k0, kw) in enumerate(KCHUNKS):
                                    lhsT = ht[:kw, c * CS : c * CS + etile]
                                    nc.tensor.matmul(
                                        ps_s1[:etile, :],
                                        lhsT,
                                        m_sb[r][:kw, c * D : c * D + D0],
                                        start=(c == 0),
                                        stop=(c == len(KCHUNKS) - 1),
                                    )
                                    nc.tensor.matmul(
                                        ps_s2[:etile, :],
                                        lhsT,
                                        m_sb[r][:kw, c * D + D0 : (c + 1) * D],
                                        start=(c == 0),
                                        stop=(c == len(KCHUNKS) - 1),
                                    )

                                if "nottr" in mode:
                                    nc.vector.tensor_copy(
                                        out=scores_sb[:etile, t : t + 1],
                                        in_=ps_s1[:etile, :1],
                                    )
                                    continue

                                # fused dot: score = rowsum(s * other), via
                                # custom-DVE TENSOR_TENSOR_REDUCE (the native
                                # ISA TTR opcode crashes this runtime)
                                acc = miscp.tile([128, 1], f32, tag="acc")
                                scr = miscp.tile([128, D0], bf16, tag="ttr_out")
                                nc.vector._custom_dve(
                                    TTR_OP,
                                    out=scr[:etile, :D0],
                                    in0=ps_s1[:etile, :],
                                    in1=osl[:, :D0],
                                    s0=0.0,
                                    s1=1.0,
                                    accum_out=acc[:etile, :],
                                )
                                nc.vector._custom_dve(
                                    TTR_OP,
                                    out=scr[:etile, :D1],
                                    in0=ps_s2[:etile, :],
                                    in1=osl[:, D0:],
                                    s0=acc[:etile, :],
                                    s1=1.0,
                                    accum_out=scores_sb[:etile, t : t + 1],
                                )

                        nc.sync.dma_start(
                            out=scores[comp, r], in_=scores_sb[:etile, :]
                        )

    nc.compile()
    return nc


def _shard_inputs(
    node_emb, rel_bf16, fake_f, pos_src, pos_dst, neg1_src, neg1_dst, neg2_src,
    n_cores=NCORES, es=ES, etile=ETILE, ntiles=NTILES, mean_rows=MEAN_ROWS,
):
    """Build per-core input maps."""
    in_maps = []
    for k in range(n_cores):
        sl = slice(k * es, (k + 1) * es)
        planes = []
        for arr in (pos_src, pos_dst, neg1_src, neg1_dst, neg2_src):
            # [R, es] -> [R, etile, ntiles] with edge (t*etile + p) at [p, t]
            p = (
                arr[:, sl]
                .reshape(arr.shape[0], ntiles, etile)
                .transpose(0, 2, 1)
                .astype(np.int32)
            )
            planes.append(p)
        idx_k = np.ascontiguousarray(np.stack(planes, axis=0))
        in_maps.append(
            {
                "node_emb": node_emb,
                "relw": rel_bf16,
                "fake": np.ascontiguousarray(fake_f[:, sl, :]),
                "idx": idx_k,
                "node_slice": np.ascontiguousarray(
                    node_emb[k * mean_rows : (k + 1) * mean_rows]
                ),
            }
        )
    return in_maps


LAST_RESULTS = None  # BassKernelResults of the most recent run (for profiling)
LAST_IN_MAPS = None  # per-core input maps of the most recent run (for benching)


def kernel(
    node_emb,
    rel,
    fake_f,
    mlp_w1,
    mlp_b1,
    mlp_w2,
    mlp_b2,
    pos_src,
    pos_dst,
    neg1_src,
    neg1_dst,
    neg2_src,
    neg2_dst,
):
    global LAST_RESULTS, LAST_IN_MAPS
    import ml_dtypes

    from concourse.bass_utils import run_bass_kernel_spmd

    node_emb = np.ascontiguousarray(np.asarray(node_emb, dtype=np.float32))
    rel = np.asarray(rel, dtype=np.float32)
    fake_f = np.asarray(fake_f, dtype=np.float32)
    rel_bf16 = rel.astype(ml_dtypes.bfloat16)

    pos_src = np.asarray(pos_src, dtype=np.int32)
    pos_dst = np.asarray(pos_dst, dtype=np.int32)
    neg1_src = np.asarray(neg1_src, dtype=np.int32)
    neg1_dst = np.asarray(neg1_dst, dtype=np.int32)
    neg2_src = np.asarray(neg2_src, dtype=np.int32)

    key = "full"
    if key not in _BUILD_CACHE:
        _BUILD_CACHE[key] = build_kernel()
    nc = _BUILD_CACHE[key]

    in_maps = _shard_inputs(
        node_emb, rel_bf16, fake_f, pos_src, pos_dst, neg1_src, neg1_dst, neg2_src
    )
    LAST_IN_MAPS = in_maps

    trace = bool(int(os.environ.get("KERNEL_TRACE", "0")))
    res = run_bass_kernel_spmd(nc, in_maps, list(range(NCORES)), trace=trace)
    LAST_RESULTS = res

    # assemble scores
    outs = {c: np.empty((R, E), np.float32) for c in range(3)}
    partial = np.zeros((D,), np.float64)
    for k in range(NCORES):
        rk = res.results[k]
        sc = rk["scores"]  # [3, R, etile, ntiles]
        for c in range(3):
            for r in range(R):
                outs[c][r, k * ES : (k + 1) * ES] = (
                    sc[c, r].T.reshape(ES).astype(np.float32)
                )
        partial += rk["mean_partial"].astype(np.float64).sum(axis=0)

    # graph embedding MLP on host (trivial size), fp32 like the reference
    hg = (partial / N_NODES).astype(np.float32)
    h = np.maximum(
        hg @ np.asarray(mlp_w1, np.float32) + np.asarray(mlp_b1, np.float32), 0.0
    )
    graph_embd = h @ np.asarray(mlp_w2, np.float32) + np.asarray(mlp_b2, np.float32)

    return (
        outs[0].reshape(-1),
        outs[1].reshape(-1),
        outs[2].reshape(-1),
        graph_embd.astype(np.float32),
    )
